# revision 40
# baseline (speedup 1.0000x reference)
"""FSQ codebook kernel for Trainium2 (8 NeuronCores, data-parallel over tokens).

Computes, for x:(8,8192,1280) f32, W:(8,1280) f32, b:(8,) f32:
    h  = x.reshape(-1,1280) @ W.T + b            # (65536, 8)
    mu = sum_k 3^k * (1 + round(tanh(h)*SCALE))  # base-3 code, int32
    -> (8, 8192) int32

round(tanh(h)*SCALE) is replaced by an exact fp32 threshold T_POS, so
digit value = [h >= T-b] + [h > -T-b] (bias folded into per-digit
threshold constants; no bias matmul).  x and W are scaled by 2^10 and
Dekker-split into fp16 hi/lo on the host; h is computed scaled by 2^20.

Phase 1 streams only the fp16 hi half of x (21 MB instead of 42 MB) at
~340 GB/s, host-pre-transposed as (dt, group, token) so chunk loads cut
along the contraction dim and the four 512-token windows of each
2-group batch run CONCURRENTLY in the four PE column groups.  A digit
can only be wrong if |h1 -+ (T-b)| < DELTA; borderline test
((h+b)^2-T^2)^2 < (2*T*DELTA)^2 via two scalar-engine Squares.

Fix-up (all device-side; host does placement only):
- round A (batches 0+1): gpsimd.sparse_gather compacts flagged ids
  while the stream still runs, so its 7.5us scan AND the ~9us mlp
  ucode-library reload it forces are hidden under the stream.
- batches 2 and 3: PE/DVE-only compaction (no sparse_gather => no
  second library reload): transpose flags to put tokens on partitions,
  rank flagged tokens within their 128-token column via a strict
  lower-triangular matmul, extract rank j of column c into static slot
  16j+c with broadcast-AP compares (relative ids + 1, fp16-exact).
  Overflow (>16 flagged in one column) is detected via the rank==16
  block and asserted zero on the host.
Each round dma_gathers the hi||lo rows of its <=256 slots and an exact
fp16x2 GEMM recomputes their digits.  Every slot's fix value is the
exact recomputation for a real token (empty slots point at a filler
token), so the host applies all of them unconditionally.
"""

import numpy as np

# exact fp32 threshold: minimal fp32 v with round(tanh(v)*SCALE) == 1
T_POS = float(np.uint32(0x3F0CCB15).view(np.float32))
SPLIT_SCALE = 1024.0  # 2^10 per operand; h is scaled by 2^20

N_CORES = 8
TOK_PER_CORE = 8192
D = 1280
K = 8
D_TILES = D // 128            # 10

GTOK = 1024
N_GROUP = TOK_PER_CORE // GTOK  # 8
NB = N_GROUP // 2               # 4 batches of 2 groups / 4 halves
NH = 2 * N_GROUP                # 16 halves of 512 tokens
HCOLS = D_TILES * GTOK          # phase-1 x cols per group

T_HI = T_POS * SPLIT_SCALE * SPLIT_SCALE
DELTA = 2.5e-3 * SPLIT_SCALE * SPLIT_SCALE        # borderline margin
FLAG_THRESH = (2.0 * T_HI * DELTA) ** 2           # on ((h+b)^2-T^2)^2
NG = 256                                          # compact slots (mult of 128)
NJ = 17                                           # 16 rank slots + overflow

# one-DMA constant blob layout (byte offsets per partition, 512-aligned)
COFF = {
    "wpk": 0, "pw4": 1024, "hselB": 1536, "hselB2": 2048, "L128": 2560,
    "jgrid": 3072, "thrPc": 4608, "thrNc": 5120, "bcol": 5632,
    "iotaw2": 6144, "iotaB": 8192, "bc16": 8704, "id16": 9216,
}
CBYTES = 10240

_cached = {}


def _build(repeat=1):
    from contextlib import ExitStack

    from concourse import bacc, mybir, tile
    from concourse.bass import AP

    f16 = mybir.dt.float16
    f32 = mybir.dt.float32
    i16 = mybir.dt.int16
    i32 = mybir.dt.int32
    u32 = mybir.dt.uint32

    nc = bacc.Bacc("TRN2", target_bir_lowering=False, debug=False)

    # x hi layout: row (gg,p), cols (dt, g2, t) -- chunk loads cut along dt
    xh_ap = nc.dram_tensor("xh", [NB * 128, 2 * HCOLS], f16, kind="ExternalInput").ap()
    xp_ap = nc.dram_tensor("xp", [TOK_PER_CORE, 2 * D], f16, kind="ExternalInput").ap()
    cblob_ap = nc.dram_tensor("cblob", [128, CBYTES], mybir.dt.uint8, kind="ExternalInput").ap()

    out_ap = nc.dram_tensor("out", [NH, 512], i32, kind="ExternalOutput").ap()
    fmuA_ap = nc.dram_tensor("fmuA", [1, NG], i32, kind="ExternalOutput").ap()
    fidxA_ap = nc.dram_tensor("fidxA", [16, NG // 16], i32, kind="ExternalOutput").ap()
    fnum_ap = nc.dram_tensor("fnum", [1, 1], u32, kind="ExternalOutput").ap()


    with tile.TileContext(nc) as tc, ExitStack() as ctx:
        const_pool = ctx.enter_context(tc.tile_pool(name="const", bufs=1))
        xt_pool = ctx.enter_context(tc.tile_pool(name="xt", bufs=3))
        val_pool = ctx.enter_context(tc.tile_pool(name="val", bufs=1))
        mu_pool = ctx.enter_context(tc.tile_pool(name="mu", bufs=1))
        fix_pool = ctx.enter_context(tc.tile_pool(name="fix", bufs=1))
        ps_h = ctx.enter_context(tc.tile_pool(name="ps_h", bufs=2, space="PSUM"))
        ps_mu = ctx.enter_context(tc.tile_pool(name="ps_mu", bufs=1, space="PSUM"))
        ps_f = ctx.enter_context(tc.tile_pool(name="ps_f", bufs=1, space="PSUM"))
        ps_2 = ctx.enter_context(tc.tile_pool(name="ps_2", bufs=1, space="PSUM"))

        # reserve PSUM pool regions upfront (pools grow lazily and the
        # late-growing pool would otherwise hit bank fragmentation)
        ps_h.tile([128, 512], f32, name="h4x")
        ps_mu.tile([128, 512], f32, name="mu4")
        ps_f.tile([16, 512], f32, name="flags_all")
        ps_2.tile([128, 512], f32, name="pA")
        ps_2.tile([128, 512], f32, name="pB")
        ps_2.tile([128, 512], f32, name="pC")

        xgs = {}

        def load_xg(gg, chunk_dts):
            # chunked loads cut along dt: every chunk covers all 4 windows,
            # so matmuls trail the stream chunk by chunk and the trailing
            # compute after the last chunk is minimal
            xg = xt_pool.tile([128, 2 * HCOLS], f16, name="xg")
            dt0 = 0
            for ndt in chunk_dts:
                c0, c1 = dt0 * 2 * GTOK, (dt0 + ndt) * 2 * GTOK
                nc.sync.dma_start(
                    xg[:, c0:c1],
                    xh_ap[gg * 128 : (gg + 1) * 128, c0:c1],
                )
                dt0 += ndt
            xgs[gg] = xg

        # the one-DMA const blob goes first on the sync ring: it completes
        # at full bandwidth before the stream saturates, freeing its
        # completion-semaphore lane immediately (a slow const DMA holding a
        # lane stalls stream-chunk generation at lane-reuse time)
        WP = 40
        blob = const_pool.tile([128, CBYTES], mybir.dt.uint8, name="cblob")
        nc.sync.dma_start(blob[:], cblob_ap[:])

        load_xg(0, [10])
        load_xg(1, [10])
        load_xg(2, [10])
        # batch 3 issued upfront too: its chunk generations must reach the
        # HWDGE ring before any tail DMA can steal a completion-semaphore
        # lane (the ring stalls harmlessly on the xg WAR while batches 1-2
        # keep the queue deep)
        load_xg(3, [5, 3, 2])

        # stacked stationary, 40 cols per d-tile: cols [0:8]=Whi_dt,
        # [32:40]=Wlo_dt.  Phase 1 uses cols [0:8]; phase 2 the full 40.
        # cols 8:32 stay uninitialized: they only feed the unread PSUM
        # rows 8:32 of the phase-2 fix GEMM.
        wpair_sb = blob[:, COFF["wpk"] : COFF["wpk"] + 800].bitcast(f16)
        pw4_sb = blob[:, COFF["pw4"] : COFF["pw4"] + 4].bitcast(f32)
        hselB_sb = blob[:, COFF["hselB"] : COFF["hselB"] + 128].bitcast(f16)
        hselB2_sb = blob[:, COFF["hselB2"] : COFF["hselB2"] + 128].bitcast(f16)
        L128_sb = blob[:, COFF["L128"] : COFF["L128"] + 256].bitcast(f16)
        jgrid_sb = blob[:, COFF["jgrid"] : COFF["jgrid"] + 1088].bitcast(f32)
        thrPc_sb = blob[:, COFF["thrPc"] : COFF["thrPc"] + 4].bitcast(f32)
        thrNc_sb = blob[:, COFF["thrNc"] : COFF["thrNc"] + 4].bitcast(f32)
        bcol_sb = blob[:, COFF["bcol"] : COFF["bcol"] + 4].bitcast(f32)
        iotaw2_sb = blob[0:16, COFF["iotaw2"] : COFF["iotaw2"] + 2048].bitcast(f32)
        iotaB_sb = blob[0:16, COFF["iotaB"] : COFF["iotaB"] + 512].bitcast(f32)
        bc16_sb = blob[0:16, COFF["bc16"] : COFF["bc16"] + 512].bitcast(f32)
        id16_sb = blob[0:16, COFF["id16"] : COFF["id16"] + 64].bitcast(f32)
        onesc = const_pool.tile([128, 1], f32)
        nc.vector.memset(onesc[:], 1.0)
        sqbias = const_pool.tile([128, 1], f32)
        nc.vector.memset(sqbias[:], -(T_HI * T_HI))

        # prime BOTH gpsimd ucode libraries during startup: dma_gather's
        # (mlp) first with a tiny 32KB gather, then sparse_gather's, so the
        # round-A tail calls pay no cold-library cost where it shows
        idxP = fix_pool.tile([128, 8], i16, name="idxP")
        nc.vector.memset(idxP[:], 0)
        gatP = fix_pool.tile([128, 1, 128], f16, name="gatP")
        nc.gpsimd.dma_gather(
            out_ap=gatP[:], in_ap=xp_ap[:, 0:128], idxs_ap=idxP[:],
            num_idxs=128, num_idxs_reg=128, elem_size=128, elem_step=2 * D,
            transpose=True,
        )
        encP = fix_pool.tile([16, 32], f32, name="encP")
        nc.vector.memset(encP[:], -1.0)
        cidxP = fix_pool.tile([16, 8], f32, name="cidxP")
        fnumP = fix_pool.tile([1, 1], u32, name="fnumP")
        nc.gpsimd.sparse_gather(cidxP[:], encP[:], num_found=fnumP[:])

        def rep_nj(ap):
            # [128, 16] -> [128, NJ, 16] with a stride-0 middle dim
            return AP(ap.tensor, ap.offset, [ap.ap[0], (0, NJ), ap.ap[1]])

        def bcast_free(ap, n):
            # [128, 1] -> [128, n] with a stride-0 free dim
            return AP(ap.tensor, ap.offset, [ap.ap[0], (0, n)])


        for _rep in range(repeat):
            # round-A flag counts: row q = half q's per-token flag counts
            flags_all = ps_f.tile([16, 512], f32, name="flags_all")
            gats = {}
            ccls = {}
            ovfs = {}
            flagsBs = {}

            def roundA_front():
                # issued right after batch 1: the sparse_gather scan AND the
                # mlp-library reload it forces both hide under the stream
                enc = fix_pool.tile([16, 512], f32, name="enc")
                nc.vector.scalar_tensor_tensor(
                    out=enc[:], in0=flags_all[:], scalar=0.0,
                    in1=iotaw2_sb[:],
                    op0=mybir.AluOpType.is_gt, op1=mybir.AluOpType.mult,
                )
                nc.vector.tensor_scalar(
                    out=enc[:], in0=enc[:], scalar1=-1.0, scalar2=None,
                    op0=mybir.AluOpType.add,
                )
                cidx = fix_pool.tile([16, NG // 16], f32, name="cidx")
                fnum = fix_pool.tile([1, 1], u32, name="fnum")
                nc.gpsimd.sparse_gather(cidx[:], enc[:], num_found=fnum[:])
                # replicate rows mod 16 across partitions with one PE
                # matmul (runs at end-of-program, PE idle, no FIFO risk)
                idxPS = ps_2.tile([128, 512], f32, name="pC")[:, 0 : NG // 16]
                nc.tensor.matmul(
                    idxPS[:], lhsT=bc16_sb[:], rhs=cidx[:], start=True, stop=True
                )
                ccl = fix_pool.tile([128, NG // 16], f32, name="ccl")
                nc.vector.tensor_scalar(
                    out=ccl[:], in0=idxPS[:], scalar1=0.0,
                    scalar2=float(TOK_PER_CORE - 1),
                    op0=mybir.AluOpType.max, op1=mybir.AluOpType.min,
                )
                ccls[0] = ccl
                idx128 = fix_pool.tile([128, NG // 16], i16, name="idx128")
                nc.vector.tensor_copy(idx128[:], ccl[:])
                gat = fix_pool.tile([128, 2 * D_TILES, NG], f16, name="gatA")
                nc.gpsimd.dma_gather(
                    out_ap=gat[:], in_ap=xp_ap[:], idxs_ap=idx128[:],
                    num_idxs=NG, num_idxs_reg=NG, elem_size=2 * D, transpose=True,
                )
                gats[0] = gat
                fnums[0] = fnum

            fnums = {}

            def compact_pe(r, flags_B, offset, iscr_ap):
                # PE/DVE-only compaction for one 2048-token batch
                encv = fix_pool.tile([16, 128], f32, name=f"encv{r}")
                nc.vector.scalar_tensor_tensor(
                    out=encv[:], in0=flags_B[:], scalar=0.0,
                    in1=iotaB_sb[:],
                    op0=mybir.AluOpType.is_gt, op1=mybir.AluOpType.mult,
                )
                encT = ps_2.tile([128, 512], f32, name="pC")[:, 0:16]
                nc.tensor.transpose(encT[:], encv[:], id16_sb[:])
                flags01 = fix_pool.tile([128, 16], f16, name=f"flags01{r}")
                nc.vector.tensor_scalar(
                    out=flags01[:], in0=encT[:], scalar1=0.5, scalar2=None,
                    op0=mybir.AluOpType.is_ge,
                )
                encTs = fix_pool.tile([128, 16], f32, name=f"encTs{r}")
                nc.vector.tensor_copy(encTs[:], encT[:])
                rank_ps = ps_2.tile([128, 512], f32, name="pC")[:, 0:16]
                nc.tensor.matmul(
                    rank_ps[:], lhsT=L128_sb[:], rhs=flags01[:],
                    start=True, stop=True,
                )
                # rank blocks 0..15 + overflow detector (rank==16) in one
                # broadcast-AP compare against the j grid
                ej = fix_pool.tile([128, NJ * 16], f32, name=f"ej{r}")
                nc.vector.tensor_tensor(
                    ej[:], rep_nj(rank_ps[:]), jgrid_sb[:],
                    mybir.AluOpType.is_equal,
                )
                nc.vector.tensor_tensor(
                    ej[:], ej[:], rep_nj(encTs[:]), mybir.AluOpType.mult,
                )
                idc = ps_2.tile([128, 512], f32, name="pB")[0:1, 0 : NJ * 16]
                nc.tensor.matmul(
                    idc[:], lhsT=onesc[:], rhs=ej[:], start=True, stop=True
                )
                idc_sb = fix_pool.tile([1, NJ * 16], f32, name=f"idc{r}")
                nc.vector.tensor_copy(idc_sb[:], idc[:])
                ovf_sb = fix_pool.tile([1, 16], i32, name=f"ovf{r}")
                nc.vector.tensor_copy(ovf_sb[:], idc_sb[:, 256 : NJ * 16])
                ovfs[r] = ovf_sb
                # [1,256] -> [16,16] partition spread (one small SB->SB DMA)
                idx16 = fix_pool.tile([16, 16], f32, name=f"idx16{r}")
                nc.scalar.dma_start(idx16[:], idc_sb[:, 0:256])
                idxPS = ps_2.tile([128, 512], f32, name="pC")[:, 0:16]
                nc.tensor.matmul(
                    idxPS[:], lhsT=bc16_sb[:], rhs=idx16[:], start=True, stop=True
                )
                ccl = fix_pool.tile([128, 16], f32, name=f"ccl{r}")
                nc.vector.tensor_scalar(
                    out=ccl[:], in0=idxPS[:], scalar1=float(offset),
                    scalar2=float(TOK_PER_CORE - 1),
                    op0=mybir.AluOpType.add, op1=mybir.AluOpType.min,
                )
                ccls[r] = ccl
                idx128 = fix_pool.tile([128, 16], i16, name=f"idx128_{r}")
                nc.vector.tensor_copy(idx128[:], ccl[:])
                gat = fix_pool.tile([128, 2 * D_TILES, NG], f16, name=f"gat{r}")
                nc.gpsimd.dma_gather(
                    out_ap=gat[:], in_ap=xp_ap[:], idxs_ap=idx128[:],
                    num_idxs=NG, num_idxs_reg=NG, elem_size=2 * D, transpose=True,
                )
                gats[r] = gat

            def do_fix_back(r, my_fmu_ap, my_fidx_ap):
                gat = gats[r]
                h40f = ps_2.tile([128, 512], f32, name="pA")[0:WP, 0:NG]
                nmm = 2 * D_TILES
                i = 0
                for dt in range(D_TILES):
                    for s in range(2):
                        nc.tensor.matmul(
                            h40f[:],
                            lhsT=wpair_sb[:, dt * WP : (dt + 1) * WP],
                            rhs=gat[:, s * D_TILES + dt, :],
                            start=(i == 0), stop=(i == nmm - 1),
                        )
                        i += 1
                hlo_sb = fix_pool.tile([K, NG], f32, name=f"hlo{r}")
                nc.vector.tensor_copy(hlo_sb[:], h40f[32 : 32 + K, :])
                hsum = fix_pool.tile([K, NG], f32, name=f"hsum{r}")
                nc.vector.tensor_add(hsum[:], h40f[0:K, :], hlo_sb[:])
                fval1 = fix_pool.tile([K, NG], f32, name=f"fval1{r}")
                nc.vector.tensor_tensor(
                    fval1[:], hsum[:], bcast_free(thrPc_sb[0:K, :], NG),
                    mybir.AluOpType.is_ge
                )
                fval2 = fix_pool.tile([K, NG], f32, name=f"fval2{r}")
                nc.vector.tensor_tensor(
                    fval2[:], hsum[:], bcast_free(thrNc_sb[0:K, :], NG),
                    mybir.AluOpType.is_gt
                )
                fval = fix_pool.tile([K, NG], f32, name=f"fval{r}")
                nc.vector.tensor_add(fval[:], fval1[:], fval2[:])
                fmu_ps = ps_2.tile([128, 512], f32, name="pB")[0:1, 0:NG]
                nc.tensor.matmul(
                    fmu_ps[:], lhsT=pw4_sb[0:K, :], rhs=fval[:], start=True, stop=True
                )
                fmu_sb = fix_pool.tile([1, NG], i32, name=f"fmu{r}")
                nc.vector.tensor_copy(fmu_sb[:], fmu_ps[:])
                nc.scalar.dma_start(my_fmu_ap[:], fmu_sb[:])
                # host-only outputs, deferred off the fix critical path
                fidx_sb = fix_pool.tile([16, NG // 16], i32, name=f"fidx{r}")
                nc.vector.tensor_copy(fidx_sb[:], ccls[r][0:16, :])
                nc.scalar.dma_start(my_fidx_ap[:], fidx_sb[:])
                nc.scalar.dma_start(fnum_ap[:], fnums[0][:])

            def do_batch(gg):
                xg = xgs[gg]

                # 4 halves concurrently in the 4 PE column groups
                h4x = ps_h.tile([128, 512], f32, name="h4x")
                for dt in range(D_TILES):
                    for j in range(4):
                        g2, hh = j // 2, j % 2
                        c0 = dt * 2 * GTOK + g2 * GTOK + hh * 512
                        nc.tensor.matmul(
                            h4x[32 * j : 32 * j + K, :],
                            lhsT=wpair_sb[:, dt * WP : dt * WP + K],
                            rhs=xg[:, c0 : c0 + 512],
                            start=(dt == 0), stop=(dt == D_TILES - 1),
                            tile_position=(0, 32 * j), skip_group_check=True,
                        )

                # batched postprocessing; the scalar-engine Squares first so
                # the flag path (sq1->sq2->flagk) never queues behind the
                # DVE value ops
                sq1 = val_pool.tile([128, 512], f32, name="sq1")
                nc.scalar.activation(
                    sq1[:], h4x[:], mybir.ActivationFunctionType.Square,
                    bias=bcol_sb[:], scale=1.0,
                )
                sq2 = val_pool.tile([128, 512], f32, name="sq2")
                nc.scalar.activation(
                    sq2[:], sq1[:], mybir.ActivationFunctionType.Square,
                    bias=sqbias[:], scale=1.0,
                )
                flagk = val_pool.tile([128, 512], f16, name="flagk")
                nc.vector.tensor_scalar(
                    out=flagk[:], in0=sq2[:], scalar1=FLAG_THRESH, scalar2=None,
                    op0=mybir.AluOpType.is_lt,
                )
                # flag-count matmul: lhsT block gg routes window j's
                # count to flags row 4gg+j
                nc.tensor.matmul(
                    flags_all[:],
                    lhsT=hselB_sb[:, gg * 16 : (gg + 1) * 16],
                    rhs=flagk[:],
                    start=(gg == 0),
                    stop=(gg == 3),
                    skip_group_check=True,
                )

                # digit values: bias folded into per-row thresholds
                val1 = val_pool.tile([128, 512], f32, name="val1")
                nc.vector.tensor_tensor(
                    val1[:], h4x[:], bcast_free(thrPc_sb[:], 512),
                    mybir.AluOpType.is_ge
                )
                val2 = val_pool.tile([128, 512], f32, name="val2")
                nc.vector.tensor_tensor(
                    val2[:], h4x[:], bcast_free(thrNc_sb[:], 512),
                    mybir.AluOpType.is_gt
                )
                val4 = val_pool.tile([128, 512], f32, name="val4")
                nc.vector.tensor_add(val4[:], val1[:], val2[:])

                # row-tiled mu matmuls: half j's code -> partition 32j
                mu4 = ps_mu.tile([128, 512], f32, name="mu4")
                for j in range(4):
                    nc.tensor.matmul(
                        mu4[32 * j : 32 * j + 1, :],
                        lhsT=pw4_sb[32 * j : 32 * j + K, :],
                        rhs=val4[32 * j : 32 * j + K, :],
                        start=True, stop=True,
                        tile_position=(32 * j, 32 * j), skip_group_check=True,
                    )
                mu_sb = mu_pool.tile([128, 512], i32, name="mu_sb")
                nc.vector.tensor_copy(mu_sb[:], mu4[:])
                nc.scalar.dma_start(
                    out_ap[4 * gg : 4 * gg + 4, :],
                    mu_sb[:].rearrange("(j r) n -> j r n", r=32)[:, 0, :],
                )

            # batch 3: issue the flag path first, compact rounds next, and
            # its (non-critical) value/mu path last, so the fix chain never
            # queues behind it
            do_batch(0)
            do_batch(1)
            do_batch(2)
            do_batch(3)
            roundA_front()
            do_fix_back(0, fmuA_ap, fidxA_ap)

    nc.compile()
    return nc


def _get_program(repeat=1):
    key = ("nc", repeat)
    if key not in _cached:
        _cached[key] = _build(repeat)
    return _cached[key]


def _split_f16(a32):
    hi = a32.astype(np.float16)
    lo = (a32 - hi.astype(np.float32)).astype(np.float16)
    return hi, lo


def make_in_maps(x, W, b):
    xf = np.ascontiguousarray(x.reshape(-1, D), dtype=np.float32)
    powers = (3.0 ** np.arange(K, dtype=np.float32)).astype(np.float32)
    ws = np.ascontiguousarray(W.T, dtype=np.float32) * np.float32(SPLIT_SCALE)
    wthi, wtlo = _split_f16(ws)
    # contiguous stationary pack [128, (dt, 40)]: cols 0:8 hi, 32:40 lo
    wpk = np.zeros((128, D_TILES * 40), dtype=np.float16)
    for dt in range(D_TILES):
        wpk[:, dt * 40 : dt * 40 + K] = wthi[dt * 128 : (dt + 1) * 128, :]
        wpk[:, dt * 40 + 32 : dt * 40 + 40] = wtlo[dt * 128 : (dt + 1) * 128, :]
    bs = b.astype(np.float32) * np.float32(SPLIT_SCALE * SPLIT_SCALE)

    pw4 = np.zeros((128, 1), dtype=np.float32)
    for j in range(4):
        pw4[32 * j : 32 * j + K, 0] = powers
    hselB = np.zeros((128, 4 * 16), dtype=np.float16)
    for gg in range(4):
        for j in range(4):
            q = 4 * gg + j
            hselB[32 * j : 32 * j + K, gg * 16 + q] = 1.0
    hselB2 = np.zeros((128, 4 * 16), dtype=np.float16)
    for bb in range(4):
        for j in range(4):
            hselB2[32 * j : 32 * j + K, bb * 16 + 4 * j + bb] = 1.0
    iotaw2 = (
        np.arange(TOK_PER_CORE, dtype=np.float32).reshape(16, 512) + 1.0
    )
    iotaB = np.zeros((16, 128), dtype=np.float32)
    for j in range(4):
        for bb in range(4):
            iotaB[4 * j + bb, :] = (
                512 * j + 128 * bb + np.arange(128, dtype=np.float32) + 1.0
            )
    bc16 = np.zeros((16, 128), dtype=np.float32)
    for p in range(128):
        bc16[p % 16, p] = 1.0
    id16 = np.eye(16, dtype=np.float32)
    L128 = np.triu(np.ones((128, 128), dtype=np.float16), 1)
    jgrid = np.zeros((128, NJ * 16), dtype=np.float32)
    for j in range(NJ):
        jgrid[:, 16 * j : 16 * j + 16] = float(j)
    thrPc = np.full((128, 1), 1e30, dtype=np.float32)
    thrNc = np.full((128, 1), 1e30, dtype=np.float32)
    bcol = np.zeros((128, 1), dtype=np.float32)
    for j in range(4):
        for k in range(K):
            thrPc[32 * j + k, 0] = np.float32(T_HI) - bs[k]
            thrNc[32 * j + k, 0] = np.float32(-T_HI) - bs[k]
            bcol[32 * j + k, 0] = bs[k]

    cblob = np.zeros((128, CBYTES), dtype=np.uint8)

    def put(name, arr):
        bv = arr.view(np.uint8).reshape(arr.shape[0], -1)
        cblob[: bv.shape[0], COFF[name] : COFF[name] + bv.shape[1]] = bv

    put("wpk", wpk)
    put("pw4", pw4)
    put("hselB", hselB)
    put("hselB2", hselB2)
    put("L128", L128)
    put("jgrid", jgrid)
    put("thrPc", thrPc)
    put("thrNc", thrNc)
    put("bcol", bcol)
    put("iotaw2", iotaw2)
    put("iotaB", iotaB)
    put("bc16", bc16)
    put("id16", id16)

    in_maps = []
    for c in range(N_CORES):
        xs = xf[c * TOK_PER_CORE : (c + 1) * TOK_PER_CORE] * np.float32(SPLIT_SCALE)
        hi, lo = _split_f16(xs)
        # xh[(gg,p), (dt,g2,t)] = hi[(2gg+g2)*GTOK+t, dt*128+p]
        xh = np.ascontiguousarray(
            hi.reshape(NB, 2, GTOK, D_TILES, 128).transpose(0, 4, 3, 1, 2)
        ).reshape(NB * 128, 2 * HCOLS)
        xp = np.ascontiguousarray(np.concatenate([hi, lo], axis=1))  # [tok, 2D]
        in_maps.append(
            {
                "xh": xh,
                "xp": xp,
                "cblob": cblob,
            }
        )
    return in_maps


def kernel(x: np.ndarray, W: np.ndarray, b: np.ndarray) -> np.ndarray:
    from concourse.bass_utils import run_bass_kernel_spmd

    nc = _get_program()

    B, T, Dx = x.shape
    assert (B * T, Dx) == (N_CORES * TOK_PER_CORE, D)
    in_maps = make_in_maps(x, W, b)
    res = run_bass_kernel_spmd(nc, in_maps, list(range(N_CORES)))
    chunks = []
    for c in range(N_CORES):
        r = res.results[c]
        mu = r["out"].reshape(-1).astype(np.int64)
        nf = int(r["fnum"].reshape(-1)[0])
        assert nf <= NG, f"core {c}: {nf} borderline tokens > NG={NG}"
        # every slot holds a clamped-valid token id whose fix value is the
        # exact recomputation for that token, so apply all of them
        # (empty/garbage slots just redundantly fix a real token)
        ids = r["fidxA"].T.reshape(-1)
        mu[ids] = r["fmuA"].reshape(-1)
        chunks.append(mu)
    return np.concatenate(chunks).reshape(B, T).astype(np.int32)


# revision 41
# speedup vs baseline: 1.1394x; 1.1394x over previous
"""FSQ codebook kernel for Trainium2 (8 NeuronCores, data-parallel over tokens).

Computes, for x:(8,8192,1280) f32, W:(8,1280) f32, b:(8,) f32:
    h  = x.reshape(-1,1280) @ W.T + b            # (65536, 8)
    mu = sum_k 3^k * (1 + round(tanh(h)*SCALE))  # base-3 code, int32
    -> (8, 8192) int32

round(tanh(h)*SCALE) is replaced by an exact fp32 threshold T_POS, so
digit value = [h >= T-b] + [h > -T-b] (bias folded into per-digit
threshold constants; no bias matmul).  x and W are scaled by 2^10 and
Dekker-split into fp16 hi/lo on the host; h is computed scaled by 2^20.

Phase 1 streams only the fp16 hi half of x (21 MB instead of 42 MB) at
~340 GB/s, host-pre-transposed as (dt, group, token) so chunk loads cut
along the contraction dim and the four 512-token windows of each
2-group batch run CONCURRENTLY in the four PE column groups.  A digit
can only be wrong if |h1 -+ (T-b)| < DELTA; borderline test
((h+b)^2-T^2)^2 < (2*T*DELTA)^2 via two scalar-engine Squares.

Fix-up (all device-side; host does placement only):
- round A (batches 0+1): gpsimd.sparse_gather compacts flagged ids
  while the stream still runs, so its 7.5us scan AND the ~9us mlp
  ucode-library reload it forces are hidden under the stream.
- batches 2 and 3: PE/DVE-only compaction (no sparse_gather => no
  second library reload): transpose flags to put tokens on partitions,
  rank flagged tokens within their 128-token column via a strict
  lower-triangular matmul, extract rank j of column c into static slot
  16j+c with broadcast-AP compares (relative ids + 1, fp16-exact).
  Overflow (>16 flagged in one column) is detected via the rank==16
  block and asserted zero on the host.
Each round dma_gathers the hi||lo rows of its <=256 slots and an exact
fp16x2 GEMM recomputes their digits.  Every slot's fix value is the
exact recomputation for a real token (empty slots point at a filler
token), so the host applies all of them unconditionally.
"""

import numpy as np

# exact fp32 threshold: minimal fp32 v with round(tanh(v)*SCALE) == 1
T_POS = float(np.uint32(0x3F0CCB15).view(np.float32))
SPLIT_SCALE = 1024.0  # 2^10 per operand; h is scaled by 2^20

N_CORES = 8
TOK_PER_CORE = 8192
D = 1280
K = 8
D_TILES = D // 128            # 10

GTOK = 1024
N_GROUP = TOK_PER_CORE // GTOK  # 8
NB = N_GROUP // 2               # 4 batches of 2 groups / 4 halves
NH = 2 * N_GROUP                # 16 halves of 512 tokens
HCOLS = D_TILES * GTOK          # phase-1 x cols per group

T_HI = T_POS * SPLIT_SCALE * SPLIT_SCALE
DELTA = 2.5e-3 * SPLIT_SCALE * SPLIT_SCALE        # borderline margin
FLAG_THRESH = (2.0 * T_HI * DELTA) ** 2           # on ((h+b)^2-T^2)^2
NG = 256                                          # compact slots (mult of 128)
NJ = 17                                           # 16 rank slots + overflow

# one-DMA constant blob layout (byte offsets per partition, 512-aligned)
COFF = {
    "wpk": 0, "pw4": 1024, "hselB": 1536, "hselB2": 2048, "L128": 2560,
    "jgrid": 3072, "thrPc": 4608, "thrNc": 5120, "bcol": 5632,
    "iotaw2": 6144, "iotaB": 8192, "bc16": 8704, "id16": 9216,
}
CBYTES = 10240

_cached = {}


def _build(repeat=1):
    from contextlib import ExitStack

    from concourse import bacc, mybir, tile
    from concourse.bass import AP

    f16 = mybir.dt.float16
    f32 = mybir.dt.float32
    i16 = mybir.dt.int16
    i32 = mybir.dt.int32
    u32 = mybir.dt.uint32

    nc = bacc.Bacc("TRN2", target_bir_lowering=False, debug=False)

    # x hi layout: row (gg,p), cols (dt, g2, t) -- chunk loads cut along dt
    xh_ap = nc.dram_tensor("xh", [NB * 128, 2 * HCOLS], f16, kind="ExternalInput").ap()
    xp_ap = nc.dram_tensor("xp", [TOK_PER_CORE, 2 * D], f16, kind="ExternalInput").ap()
    cblob_ap = nc.dram_tensor("cblob", [128, CBYTES], mybir.dt.uint8, kind="ExternalInput").ap()

    out_ap = nc.dram_tensor("out", [NH, 512], i32, kind="ExternalOutput").ap()
    fmuA_ap = nc.dram_tensor("fmuA", [1, NG], i32, kind="ExternalOutput").ap()
    fidxA_ap = nc.dram_tensor("fidxA", [16, NG // 16], i32, kind="ExternalOutput").ap()
    fnum_ap = nc.dram_tensor("fnum", [1, 1], u32, kind="ExternalOutput").ap()


    with tile.TileContext(nc) as tc, ExitStack() as ctx:
        const_pool = ctx.enter_context(tc.tile_pool(name="const", bufs=1))
        xt_pool = ctx.enter_context(tc.tile_pool(name="xt", bufs=3))
        val_pool = ctx.enter_context(tc.tile_pool(name="val", bufs=1))
        mu_pool = ctx.enter_context(tc.tile_pool(name="mu", bufs=1))
        fix_pool = ctx.enter_context(tc.tile_pool(name="fix", bufs=1))
        ps_h = ctx.enter_context(tc.tile_pool(name="ps_h", bufs=2, space="PSUM"))
        ps_mu = ctx.enter_context(tc.tile_pool(name="ps_mu", bufs=1, space="PSUM"))
        ps_f = ctx.enter_context(tc.tile_pool(name="ps_f", bufs=1, space="PSUM"))
        ps_2 = ctx.enter_context(tc.tile_pool(name="ps_2", bufs=1, space="PSUM"))

        # reserve PSUM pool regions upfront (pools grow lazily and the
        # late-growing pool would otherwise hit bank fragmentation)
        ps_h.tile([128, 512], f32, name="h4x")
        ps_mu.tile([128, 512], f32, name="mu4")
        ps_f.tile([16, 512], f32, name="flags_all")
        ps_2.tile([128, 512], f32, name="pA")
        ps_2.tile([128, 512], f32, name="pB")
        ps_2.tile([128, 512], f32, name="pC")

        xgs = {}

        def load_xg(gg, chunk_dts):
            # chunked loads cut along dt: every chunk covers all 4 windows,
            # so matmuls trail the stream chunk by chunk and the trailing
            # compute after the last chunk is minimal
            xg = xt_pool.tile([128, 2 * HCOLS], f16, name="xg")
            dt0 = 0
            for ndt in chunk_dts:
                c0, c1 = dt0 * 2 * GTOK, (dt0 + ndt) * 2 * GTOK
                nc.sync.dma_start(
                    xg[:, c0:c1],
                    xh_ap[gg * 128 : (gg + 1) * 128, c0:c1],
                )
                dt0 += ndt
            xgs[gg] = xg

        # the one-DMA const blob goes first on the sync ring: it completes
        # at full bandwidth before the stream saturates, freeing its
        # completion-semaphore lane immediately (a slow const DMA holding a
        # lane stalls stream-chunk generation at lane-reuse time)
        WP = 40
        blob = const_pool.tile([128, CBYTES], mybir.dt.uint8, name="cblob")
        nc.sync.dma_start(blob[:], cblob_ap[:])

        load_xg(0, [10])
        load_xg(1, [10])
        load_xg(2, [10])
        # batch 3 issued upfront too: its chunk generations must reach the
        # HWDGE ring before any tail DMA can steal a completion-semaphore
        # lane (the ring stalls harmlessly on the xg WAR while batches 1-2
        # keep the queue deep)
        load_xg(3, [5, 3, 2])

        # stacked stationary, 40 cols per d-tile: cols [0:8]=Whi_dt,
        # [32:40]=Wlo_dt.  Phase 1 uses cols [0:8]; phase 2 the full 40.
        # cols 8:32 stay uninitialized: they only feed the unread PSUM
        # rows 8:32 of the phase-2 fix GEMM.
        wpair_sb = blob[:, COFF["wpk"] : COFF["wpk"] + 800].bitcast(f16)
        pw4_sb = blob[:, COFF["pw4"] : COFF["pw4"] + 4].bitcast(f32)
        hselB_sb = blob[:, COFF["hselB"] : COFF["hselB"] + 128].bitcast(f16)
        hselB2_sb = blob[:, COFF["hselB2"] : COFF["hselB2"] + 128].bitcast(f16)
        L128_sb = blob[:, COFF["L128"] : COFF["L128"] + 256].bitcast(f16)
        jgrid_sb = blob[:, COFF["jgrid"] : COFF["jgrid"] + 1088].bitcast(f32)
        thrPc_sb = blob[:, COFF["thrPc"] : COFF["thrPc"] + 4].bitcast(f32)
        thrNc_sb = blob[:, COFF["thrNc"] : COFF["thrNc"] + 4].bitcast(f32)
        bcol_sb = blob[:, COFF["bcol"] : COFF["bcol"] + 4].bitcast(f32)
        iotaw2_sb = blob[0:16, COFF["iotaw2"] : COFF["iotaw2"] + 2048].bitcast(f32)
        iotaB_sb = blob[0:16, COFF["iotaB"] : COFF["iotaB"] + 512].bitcast(f32)
        bc16_sb = blob[0:16, COFF["bc16"] : COFF["bc16"] + 512].bitcast(f32)
        id16_sb = blob[0:16, COFF["id16"] : COFF["id16"] + 64].bitcast(f32)
        onesc = const_pool.tile([128, 1], f32)
        nc.vector.memset(onesc[:], 1.0)
        sqbias = const_pool.tile([128, 1], f32)
        nc.vector.memset(sqbias[:], -(T_HI * T_HI))

        # prime BOTH gpsimd ucode libraries during startup: dma_gather's
        # (mlp) first with a tiny 32KB gather, then sparse_gather's, so the
        # round-A tail calls pay no cold-library cost where it shows
        idxP = fix_pool.tile([128, 8], i16, name="idxP")
        nc.vector.memset(idxP[:], 0)
        gatP = fix_pool.tile([128, 1, 128], f16, name="gatP")
        nc.gpsimd.dma_gather(
            out_ap=gatP[:], in_ap=xp_ap[:, 0:128], idxs_ap=idxP[:],
            num_idxs=128, num_idxs_reg=128, elem_size=128, elem_step=2 * D,
            transpose=True,
        )
        encP = fix_pool.tile([16, 32], f32, name="encP")
        nc.vector.memset(encP[:], -1.0)
        cidxP = fix_pool.tile([16, 8], f32, name="cidxP")
        fnumP = fix_pool.tile([1, 1], u32, name="fnumP")
        nc.gpsimd.sparse_gather(cidxP[:], encP[:], num_found=fnumP[:])

        def rep_nj(ap):
            # [128, 16] -> [128, NJ, 16] with a stride-0 middle dim
            return AP(ap.tensor, ap.offset, [ap.ap[0], (0, NJ), ap.ap[1]])

        def bcast_free(ap, n):
            # [128, 1] -> [128, n] with a stride-0 free dim
            return AP(ap.tensor, ap.offset, [ap.ap[0], (0, n)])


        for _rep in range(repeat):
            # round-A flag counts: row q = half q's per-token flag counts
            flags_all = ps_f.tile([16, 512], f32, name="flags_all")
            gats = {}
            ccls = {}
            ovfs = {}
            flagsBs = {}

            def roundA_front():
                # issued right after batch 1: the sparse_gather scan AND the
                # mlp-library reload it forces both hide under the stream
                enc = fix_pool.tile([16, 512], f32, name="enc")
                nc.vector.scalar_tensor_tensor(
                    out=enc[:], in0=flags_all[:], scalar=0.0,
                    in1=iotaw2_sb[:],
                    op0=mybir.AluOpType.is_gt, op1=mybir.AluOpType.mult,
                )
                nc.vector.tensor_scalar(
                    out=enc[:], in0=enc[:], scalar1=-1.0, scalar2=None,
                    op0=mybir.AluOpType.add,
                )
                cidx = fix_pool.tile([16, NG // 16], f32, name="cidx")
                fnum = fix_pool.tile([1, 1], u32, name="fnum")
                nc.gpsimd.sparse_gather(cidx[:], enc[:], num_found=fnum[:])
                # replicate rows mod 16 across partitions with one PE
                # matmul (runs at end-of-program, PE idle, no FIFO risk)
                idxPS = ps_2.tile([128, 512], f32, name="pC")[:, 0 : NG // 16]
                nc.tensor.matmul(
                    idxPS[:], lhsT=bc16_sb[:], rhs=cidx[:], start=True, stop=True
                )
                ccl = fix_pool.tile([128, NG // 16], f32, name="ccl")
                nc.vector.tensor_scalar(
                    out=ccl[:], in0=idxPS[:], scalar1=0.0,
                    scalar2=float(TOK_PER_CORE - 1),
                    op0=mybir.AluOpType.max, op1=mybir.AluOpType.min,
                )
                ccls[0] = ccl
                idx128 = fix_pool.tile([128, NG // 16], i16, name="idx128")
                nc.vector.tensor_copy(idx128[:], ccl[:])
                gat = fix_pool.tile([128, 2 * D_TILES, NG], f16, name="gatA")
                nc.gpsimd.dma_gather(
                    out_ap=gat[:], in_ap=xp_ap[:], idxs_ap=idx128[:],
                    num_idxs=NG, num_idxs_reg=NG, elem_size=2 * D, transpose=True,
                )
                gats[0] = gat
                fnums[0] = fnum

            fnums = {}

            def compact_pe(r, flags_B, offset, iscr_ap):
                # PE/DVE-only compaction for one 2048-token batch
                encv = fix_pool.tile([16, 128], f32, name=f"encv{r}")
                nc.vector.scalar_tensor_tensor(
                    out=encv[:], in0=flags_B[:], scalar=0.0,
                    in1=iotaB_sb[:],
                    op0=mybir.AluOpType.is_gt, op1=mybir.AluOpType.mult,
                )
                encT = ps_2.tile([128, 512], f32, name="pC")[:, 0:16]
                nc.tensor.transpose(encT[:], encv[:], id16_sb[:])
                flags01 = fix_pool.tile([128, 16], f16, name=f"flags01{r}")
                nc.vector.tensor_scalar(
                    out=flags01[:], in0=encT[:], scalar1=0.5, scalar2=None,
                    op0=mybir.AluOpType.is_ge,
                )
                encTs = fix_pool.tile([128, 16], f32, name=f"encTs{r}")
                nc.vector.tensor_copy(encTs[:], encT[:])
                rank_ps = ps_2.tile([128, 512], f32, name="pC")[:, 0:16]
                nc.tensor.matmul(
                    rank_ps[:], lhsT=L128_sb[:], rhs=flags01[:],
                    start=True, stop=True,
                )
                # rank blocks 0..15 + overflow detector (rank==16) in one
                # broadcast-AP compare against the j grid
                ej = fix_pool.tile([128, NJ * 16], f32, name=f"ej{r}")
                nc.vector.tensor_tensor(
                    ej[:], rep_nj(rank_ps[:]), jgrid_sb[:],
                    mybir.AluOpType.is_equal,
                )
                nc.vector.tensor_tensor(
                    ej[:], ej[:], rep_nj(encTs[:]), mybir.AluOpType.mult,
                )
                idc = ps_2.tile([128, 512], f32, name="pB")[0:1, 0 : NJ * 16]
                nc.tensor.matmul(
                    idc[:], lhsT=onesc[:], rhs=ej[:], start=True, stop=True
                )
                idc_sb = fix_pool.tile([1, NJ * 16], f32, name=f"idc{r}")
                nc.vector.tensor_copy(idc_sb[:], idc[:])
                ovf_sb = fix_pool.tile([1, 16], i32, name=f"ovf{r}")
                nc.vector.tensor_copy(ovf_sb[:], idc_sb[:, 256 : NJ * 16])
                ovfs[r] = ovf_sb
                # [1,256] -> [16,16] partition spread (one small SB->SB DMA)
                idx16 = fix_pool.tile([16, 16], f32, name=f"idx16{r}")
                nc.scalar.dma_start(idx16[:], idc_sb[:, 0:256])
                idxPS = ps_2.tile([128, 512], f32, name="pC")[:, 0:16]
                nc.tensor.matmul(
                    idxPS[:], lhsT=bc16_sb[:], rhs=idx16[:], start=True, stop=True
                )
                ccl = fix_pool.tile([128, 16], f32, name=f"ccl{r}")
                nc.vector.tensor_scalar(
                    out=ccl[:], in0=idxPS[:], scalar1=float(offset),
                    scalar2=float(TOK_PER_CORE - 1),
                    op0=mybir.AluOpType.add, op1=mybir.AluOpType.min,
                )
                ccls[r] = ccl
                idx128 = fix_pool.tile([128, 16], i16, name=f"idx128_{r}")
                nc.vector.tensor_copy(idx128[:], ccl[:])
                gat = fix_pool.tile([128, 2 * D_TILES, NG], f16, name=f"gat{r}")
                nc.gpsimd.dma_gather(
                    out_ap=gat[:], in_ap=xp_ap[:], idxs_ap=idx128[:],
                    num_idxs=NG, num_idxs_reg=NG, elem_size=2 * D, transpose=True,
                )
                gats[r] = gat

            def do_fix_back(r, my_fmu_ap, my_fidx_ap):
                gat = gats[r]
                h40f = ps_2.tile([128, 512], f32, name="pA")[0:WP, 0:NG]
                nmm = 2 * D_TILES
                i = 0
                for dt in range(D_TILES):
                    for s in range(2):
                        nc.tensor.matmul(
                            h40f[:],
                            lhsT=wpair_sb[:, dt * WP : (dt + 1) * WP],
                            rhs=gat[:, s * D_TILES + dt, :],
                            start=(i == 0), stop=(i == nmm - 1),
                        )
                        i += 1
                hlo_sb = fix_pool.tile([K, NG], f32, name=f"hlo{r}")
                nc.vector.tensor_copy(hlo_sb[:], h40f[32 : 32 + K, :])
                hsum = fix_pool.tile([K, NG], f32, name=f"hsum{r}")
                nc.vector.tensor_add(hsum[:], h40f[0:K, :], hlo_sb[:])
                fval1 = fix_pool.tile([K, NG], f32, name=f"fval1{r}")
                nc.vector.tensor_tensor(
                    fval1[:], hsum[:], bcast_free(thrPc_sb[0:K, :], NG),
                    mybir.AluOpType.is_ge
                )
                fval2 = fix_pool.tile([K, NG], f32, name=f"fval2{r}")
                nc.vector.tensor_tensor(
                    fval2[:], hsum[:], bcast_free(thrNc_sb[0:K, :], NG),
                    mybir.AluOpType.is_gt
                )
                fval = fix_pool.tile([K, NG], f32, name=f"fval{r}")
                nc.vector.tensor_add(fval[:], fval1[:], fval2[:])
                fmu_ps = ps_2.tile([128, 512], f32, name="pB")[0:1, 0:NG]
                nc.tensor.matmul(
                    fmu_ps[:], lhsT=pw4_sb[0:K, :], rhs=fval[:], start=True, stop=True
                )
                fmu_sb = fix_pool.tile([1, NG], i32, name=f"fmu{r}")
                nc.vector.tensor_copy(fmu_sb[:], fmu_ps[:])
                nc.scalar.dma_start(my_fmu_ap[:], fmu_sb[:])
                # host-only outputs, deferred off the fix critical path
                fidx_sb = fix_pool.tile([16, NG // 16], i32, name=f"fidx{r}")
                nc.vector.tensor_copy(fidx_sb[:], ccls[r][0:16, :])
                nc.scalar.dma_start(my_fidx_ap[:], fidx_sb[:])
                nc.scalar.dma_start(fnum_ap[:], fnums[0][:])

            def do_batch(gg, hook=None):
                xg = xgs[gg]

                # 4 halves concurrently in the 4 PE column groups
                h4x = ps_h.tile([128, 512], f32, name="h4x")
                for dt in range(D_TILES):
                    for j in range(4):
                        g2, hh = j // 2, j % 2
                        c0 = dt * 2 * GTOK + g2 * GTOK + hh * 512
                        nc.tensor.matmul(
                            h4x[32 * j : 32 * j + K, :],
                            lhsT=wpair_sb[:, dt * WP : dt * WP + K],
                            rhs=xg[:, c0 : c0 + 512],
                            start=(dt == 0), stop=(dt == D_TILES - 1),
                            tile_position=(0, 32 * j), skip_group_check=True,
                        )

                # batched postprocessing; the scalar-engine Squares first so
                # the flag path (sq1->sq2->flagk) never queues behind the
                # DVE value ops
                sq1 = val_pool.tile([128, 512], f32, name="sq1")
                nc.scalar.activation(
                    sq1[:], h4x[:], mybir.ActivationFunctionType.Square,
                    bias=bcol_sb[:], scale=1.0,
                )
                sq2 = val_pool.tile([128, 512], f32, name="sq2")
                nc.scalar.activation(
                    sq2[:], sq1[:], mybir.ActivationFunctionType.Square,
                    bias=sqbias[:], scale=1.0,
                )
                flagk = val_pool.tile([128, 512], f16, name="flagk")
                nc.vector.tensor_scalar(
                    out=flagk[:], in0=sq2[:], scalar1=FLAG_THRESH, scalar2=None,
                    op0=mybir.AluOpType.is_lt,
                )
                # flag-count matmul: lhsT block gg routes window j's
                # count to flags row 4gg+j
                nc.tensor.matmul(
                    flags_all[:],
                    lhsT=hselB_sb[:, gg * 16 : (gg + 1) * 16],
                    rhs=flagk[:],
                    start=(gg == 0),
                    stop=(gg == 3),
                    skip_group_check=True,
                )

                if hook is not None:
                    hook()
                # digit values: bias folded into per-row thresholds
                val1 = val_pool.tile([128, 512], f32, name="val1")
                nc.vector.tensor_tensor(
                    val1[:], h4x[:], bcast_free(thrPc_sb[:], 512),
                    mybir.AluOpType.is_ge
                )
                val2 = val_pool.tile([128, 512], f32, name="val2")
                nc.vector.tensor_tensor(
                    val2[:], h4x[:], bcast_free(thrNc_sb[:], 512),
                    mybir.AluOpType.is_gt
                )
                val4 = val_pool.tile([128, 512], f32, name="val4")
                nc.vector.tensor_add(val4[:], val1[:], val2[:])

                # row-tiled mu matmuls: half j's code -> partition 32j
                mu4 = ps_mu.tile([128, 512], f32, name="mu4")
                for j in range(4):
                    nc.tensor.matmul(
                        mu4[32 * j : 32 * j + 1, :],
                        lhsT=pw4_sb[32 * j : 32 * j + K, :],
                        rhs=val4[32 * j : 32 * j + K, :],
                        start=True, stop=True,
                        tile_position=(32 * j, 32 * j), skip_group_check=True,
                    )
                mu_sb = mu_pool.tile([128, 512], i32, name="mu_sb")
                nc.vector.tensor_copy(mu_sb[:], mu4[:])
                nc.scalar.dma_start(
                    out_ap[4 * gg : 4 * gg + 4, :],
                    mu_sb[:].rearrange("(j r) n -> j r n", r=32)[:, 0, :],
                )

            # batch 3: issue the flag path first, compact rounds next, and
            # its (non-critical) value/mu path last, so the fix chain never
            # queues behind it
            do_batch(0)
            do_batch(1)
            do_batch(2)
            do_batch(3, hook=roundA_front)
            do_fix_back(0, fmuA_ap, fidxA_ap)

    nc.compile()
    return nc


def _get_program(repeat=1):
    key = ("nc", repeat)
    if key not in _cached:
        _cached[key] = _build(repeat)
    return _cached[key]


def _split_f16(a32):
    hi = a32.astype(np.float16)
    lo = (a32 - hi.astype(np.float32)).astype(np.float16)
    return hi, lo


def make_in_maps(x, W, b):
    xf = np.ascontiguousarray(x.reshape(-1, D), dtype=np.float32)
    powers = (3.0 ** np.arange(K, dtype=np.float32)).astype(np.float32)
    ws = np.ascontiguousarray(W.T, dtype=np.float32) * np.float32(SPLIT_SCALE)
    wthi, wtlo = _split_f16(ws)
    # contiguous stationary pack [128, (dt, 40)]: cols 0:8 hi, 32:40 lo
    wpk = np.zeros((128, D_TILES * 40), dtype=np.float16)
    for dt in range(D_TILES):
        wpk[:, dt * 40 : dt * 40 + K] = wthi[dt * 128 : (dt + 1) * 128, :]
        wpk[:, dt * 40 + 32 : dt * 40 + 40] = wtlo[dt * 128 : (dt + 1) * 128, :]
    bs = b.astype(np.float32) * np.float32(SPLIT_SCALE * SPLIT_SCALE)

    pw4 = np.zeros((128, 1), dtype=np.float32)
    for j in range(4):
        pw4[32 * j : 32 * j + K, 0] = powers
    hselB = np.zeros((128, 4 * 16), dtype=np.float16)
    for gg in range(4):
        for j in range(4):
            q = 4 * gg + j
            hselB[32 * j : 32 * j + K, gg * 16 + q] = 1.0
    hselB2 = np.zeros((128, 4 * 16), dtype=np.float16)
    for bb in range(4):
        for j in range(4):
            hselB2[32 * j : 32 * j + K, bb * 16 + 4 * j + bb] = 1.0
    iotaw2 = (
        np.arange(TOK_PER_CORE, dtype=np.float32).reshape(16, 512) + 1.0
    )
    iotaB = np.zeros((16, 128), dtype=np.float32)
    for j in range(4):
        for bb in range(4):
            iotaB[4 * j + bb, :] = (
                512 * j + 128 * bb + np.arange(128, dtype=np.float32) + 1.0
            )
    bc16 = np.zeros((16, 128), dtype=np.float32)
    for p in range(128):
        bc16[p % 16, p] = 1.0
    id16 = np.eye(16, dtype=np.float32)
    L128 = np.triu(np.ones((128, 128), dtype=np.float16), 1)
    jgrid = np.zeros((128, NJ * 16), dtype=np.float32)
    for j in range(NJ):
        jgrid[:, 16 * j : 16 * j + 16] = float(j)
    thrPc = np.full((128, 1), 1e30, dtype=np.float32)
    thrNc = np.full((128, 1), 1e30, dtype=np.float32)
    bcol = np.zeros((128, 1), dtype=np.float32)
    for j in range(4):
        for k in range(K):
            thrPc[32 * j + k, 0] = np.float32(T_HI) - bs[k]
            thrNc[32 * j + k, 0] = np.float32(-T_HI) - bs[k]
            bcol[32 * j + k, 0] = bs[k]

    cblob = np.zeros((128, CBYTES), dtype=np.uint8)

    def put(name, arr):
        bv = arr.view(np.uint8).reshape(arr.shape[0], -1)
        cblob[: bv.shape[0], COFF[name] : COFF[name] + bv.shape[1]] = bv

    put("wpk", wpk)
    put("pw4", pw4)
    put("hselB", hselB)
    put("hselB2", hselB2)
    put("L128", L128)
    put("jgrid", jgrid)
    put("thrPc", thrPc)
    put("thrNc", thrNc)
    put("bcol", bcol)
    put("iotaw2", iotaw2)
    put("iotaB", iotaB)
    put("bc16", bc16)
    put("id16", id16)

    in_maps = []
    for c in range(N_CORES):
        xs = xf[c * TOK_PER_CORE : (c + 1) * TOK_PER_CORE] * np.float32(SPLIT_SCALE)
        hi, lo = _split_f16(xs)
        # xh[(gg,p), (dt,g2,t)] = hi[(2gg+g2)*GTOK+t, dt*128+p]
        xh = np.ascontiguousarray(
            hi.reshape(NB, 2, GTOK, D_TILES, 128).transpose(0, 4, 3, 1, 2)
        ).reshape(NB * 128, 2 * HCOLS)
        xp = np.ascontiguousarray(np.concatenate([hi, lo], axis=1))  # [tok, 2D]
        in_maps.append(
            {
                "xh": xh,
                "xp": xp,
                "cblob": cblob,
            }
        )
    return in_maps


def kernel(x: np.ndarray, W: np.ndarray, b: np.ndarray) -> np.ndarray:
    from concourse.bass_utils import run_bass_kernel_spmd

    nc = _get_program()

    B, T, Dx = x.shape
    assert (B * T, Dx) == (N_CORES * TOK_PER_CORE, D)
    in_maps = make_in_maps(x, W, b)
    res = run_bass_kernel_spmd(nc, in_maps, list(range(N_CORES)))
    chunks = []
    for c in range(N_CORES):
        r = res.results[c]
        mu = r["out"].reshape(-1).astype(np.int64)
        nf = int(r["fnum"].reshape(-1)[0])
        assert nf <= NG, f"core {c}: {nf} borderline tokens > NG={NG}"
        # every slot holds a clamped-valid token id whose fix value is the
        # exact recomputation for that token, so apply all of them
        # (empty/garbage slots just redundantly fix a real token)
        ids = r["fidxA"].T.reshape(-1)
        mu[ids] = r["fmuA"].reshape(-1)
        chunks.append(mu)
    return np.concatenate(chunks).reshape(B, T).astype(np.int32)
